# revision 38
# baseline (speedup 1.0000x reference)
"""CRF Viterbi decode kernel for Trainium2 (8 NeuronCores, data-parallel).

Problem: B=1024, S=512, TAGSET=50 (T=52 incl START/STOP).
Strategy (fp16 relative-alpha forward + slim u32 traceback):
  - Shard batch across 8 cores (128 batches/core = 128 SBUF partitions).
  - Forward pass entirely in fp16 on the DVE (2x dual-pump throughput for
    tensor_tensor vs fp32): alpha is kept RELATIVE to its per-batch running
    max, re-centered every RC=8 steps, so values stay in fp16's precise
    range (~[-9, 22], quantization ~1e-2 worst case).  The decode argmax is
    invariant to per-(batch,t) constant shifts; measured decode rel-err
    2.9e-3 against the exact reference (budget 2e-2).
      s16[j,i] = prev[i] + T16[j,i]      fp16 tensor_tensor, 2500/partition
      h16      = max(s16[...,:25], s16[...,25:])   (reduce runs 1 elem/cyc,
      red16[j] = max_i h16[j,i]                     tt-max runs 2/cyc, so
      ahist_t  = red16 + f16_t                      pre-combining halves it)
      every RC steps: dm = max_j ahist_t; prev = ahist_t - dm
    ahist rows are stored with a 52-element stride (4B-aligned slices).
  - Best-last candidates per t vectorized in fp16 (first-index argmax via
    the (iota-1024)-min trick); chunks below the minimum length are skipped.
    The scheduler interleaves this into forward-pass DMA bubbles.
  - Traceback: sequential pointer chase, one u32 ptr slot group per step
    (decall8[:, 8t]): one-hot = tensor_tensor is_equal(iota_u32, ptr
    broadcast) -> PE transpose -> one bf16 matmul gathers T[:, ptr] (bf16
    error ~2e-3 is within the noise floor) -> s = ahist[t-1] + tcol ->
    MAX8 + FIND_INDEX8 write the argmax ptr directly into the next step's
    slot.  The baseline's f-value gather is dropped (adding a per-column
    constant cannot change the argmax), and the length-reset predicated
    copy is skipped below the minimum sequence length.
"""
import sys
import types

import numpy as np

import concourse.bass as bass
import concourse.tile as tile
from concourse import mybir
from concourse.bass_utils import run_bass_kernel_spmd


def _ensure_ntff_hook():
    """The agent image's antenv lacks axon_hooks; shim it so trace=True can
    collect NTFF profiles via the ctypes hook in trn_agent_boot."""
    try:
        from antenv.axon_hooks import get_axon_ntff_profile_hook  # noqa: F401
        return
    except ImportError:
        pass
    try:
        import trn_agent_boot.trn_boot as tb
        mod = types.ModuleType('antenv.axon_hooks')
        _h = [None]
        mod.set_axon_ntff_profile_hook = lambda h: _h.__setitem__(0, h)
        mod.get_axon_ntff_profile_hook = lambda: _h[0]
        sys.modules['antenv.axon_hooks'] = mod
        mod.set_axon_ntff_profile_hook(
            tb._ntff_profile_via_ctypes('/opt/axon/libaxon_pjrt.so'))
    except Exception:
        pass


_ensure_ntff_hook()

F32 = mybir.dt.float32
F16 = mybir.dt.float16
BF16 = mybir.dt.bfloat16
I32 = mybir.dt.int32
I8 = mybir.dt.int8

# Problem constants (hardcoded per the harness contract).
B, S, TFULL = 1024, 512, 52
NT = 50                     # real tags; START/STOP can never win (margin ~1e4)
START, STOP = 50, 51
NCORES = 8
BL = B // NCORES            # 128 batches per core = 128 partitions
BIGF = 1024.0               # iota offset for first-index argmin trick (fp16-exact)
FCH = 64 # feats DMA chunk (timesteps per DMA)

_AluOp = mybir.AluOpType
_Axis = mybir.AxisListType

_SPLICE_N = [0]
_DEBUG_DUMP = False


def _split_waits(nc, max_waits=1):
    """This walrus build encodes at most one sync wait per instruction; hoist
    extra waits onto injected same-engine NoOps (engine queues are in-order,
    so semantics are preserved)."""
    for f in nc.m.functions:
        for b in f.blocks:
            insts = b.instructions
            i = 0
            while i < len(insts):
                inst = insts[i]
                si = inst.sync_info
                waits = list(si.on_wait) if si is not None and si.on_wait else []
                if len(waits) > max_waits:
                    si.on_wait = waits[-max_waits:]
                    for w in waits[:-max_waits]:
                        _SPLICE_N[0] += 1
                        nop = mybir.InstNoOp(name=f"I-wsplit{_SPLICE_N[0]}")
                        nop.engine = inst.engine
                        nop.sync_info = mybir.SyncInfo(on_wait=[w], on_update=[])
                        insts.insert(i, nop)
                        i += 1
                i += 1


def _build_program(s_len, tmin):
    """Build the per-core Bass program. Identical on all cores (SPMD)."""
    nc = bass.Bass('TRN2', target_bir_lowering=False, debug=False)

    ftime_d = nc.dram_tensor('ftime', [BL, s_len * NT], F16, kind='ExternalInput').ap()
    rel0_d = nc.dram_tensor('rel0', [BL, NT], F16, kind='ExternalInput').ap()
    eqt8_d = nc.dram_tensor('eqt8', [BL, s_len], I8, kind='ExternalInput').ap()
    actf_d = nc.dram_tensor('actf', [BL, s_len], F32, kind='ExternalInput').ap()
    trep_d = nc.dram_tensor('trep', [BL, NT * NT], F16, kind='ExternalInput').ap()
    tstop_d = nc.dram_tensor('tstop', [BL, NT], F16, kind='ExternalInput').ap()
    iota_d = nc.dram_tensor('iotau', [BL, NT], mybir.dt.uint32,
                            kind='ExternalInput').ap()
    iota16_d = nc.dram_tensor('iotamb16', [BL, NT], F16, kind='ExternalInput').ap()
    ident_d = nc.dram_tensor('ident', [BL, BL], BF16, kind='ExternalInput').ap()
    tbf_d = nc.dram_tensor('tbf', [NT, NT], BF16, kind='ExternalInput').ap()
    dec_d = nc.dram_tensor('dec', [BL, s_len], I32, kind='ExternalOutput').ap()
    dbga_d = nc.dram_tensor('dbga', [BL, s_len * NT], F16,
                            kind='ExternalOutput').ap() if _DEBUG_DUMP else None
    dbgc_d = nc.dram_tensor('dbgc', [BL, s_len], F32,
                            kind='ExternalOutput').ap() if _DEBUG_DUMP else None

    with tile.TileContext(nc) as tc:
        with tc.tile_pool(name='res', bufs=1) as res, \
             tc.tile_pool(name='fch', bufs=4) as fpool, \
             tc.tile_pool(name='cbtmp', bufs=3) as cbpool, \
             tc.tile_pool(name='tmp', bufs=4) as tmp, \
             tc.tile_pool(name='ps', bufs=4, space='PSUM') as psum:

            # ---- resident constants & state ----
            trep = res.tile([BL, NT * NT], F16, tag='trep')
            nc.gpsimd.dma_start(trep[:], trep_d[:])
            tstop = res.tile([BL, NT], F16, tag='tstop')
            nc.gpsimd.dma_start(tstop[:], tstop_d[:])
            iota = res.tile([BL, NT], mybir.dt.uint32, tag='iota')
            nc.gpsimd.dma_start(iota[:], iota_d[:])
            iota16 = res.tile([BL, NT], F16, tag='iota16')
            nc.gpsimd.dma_start(iota16[:], iota16_d[:])
            ident = res.tile([BL, BL], BF16, tag='ident')
            nc.gpsimd.dma_start(ident[:], ident_d[:])
            tbf = res.tile([NT, NT], BF16, tag='tbf')
            nc.gpsimd.dma_start(tbf[:], tbf_d[:])
            eqt8 = res.tile([BL, s_len], I8, tag='eqt8')
            nc.gpsimd.dma_start(eqt8[:], eqt8_d[:])
            actf = res.tile([BL, s_len], F32, tag='actf')
            nc.gpsimd.dma_start(actf[:], actf_d[:])

            AST = 52                       # padded step stride (4B-aligned)
            ahist = res.tile([BL, s_len * AST], F16, tag='ahist')
            nc.gpsimd.dma_start(ahist[:, 0:NT], rel0_d[:])

            s16 = res.tile([BL, NT * NT], F16, tag='s16')
            h16 = res.tile([BL, NT * 25], F16, tag='h16')
            g16 = res.tile([BL, 600], F16, tag='g16')
            rel16 = res.tile([BL, NT], F16, tag='rel16')
            nc.vector.tensor_copy(rel16[:], ahist[:, 0:NT])
            dm = res.tile([BL, 1], F32, tag='dm')
            decall8 = res.tile([BL, s_len * 8], mybir.dt.uint32,
                               tag='decall8')
            nc.vector.memset(decall8[:], 0.0)
            cballu = res.tile([BL, s_len], mybir.dt.uint32, tag='cballu')
            cball16 = res.tile([BL, s_len], F16, tag='cball16')
            mall16 = res.tile([BL, s_len], F16, tag='mall16')

            # ---- forward (all fp16 on DVE; re-center every RC steps) ----
            RC = 8
            fwd_scope = nc.named_scope('fwd')
            fwd_scope.__enter__()
            n_ch = (s_len + FCH - 1) // FCH
            for c in range(n_ch):
                t0 = c * FCH
                t1 = min(t0 + FCH, s_len)
                ft = fpool.tile([BL, (t1 - t0) * NT], F16, tag='fch')
                nc.gpsimd.dma_start(ft[:], ftime_d[:, t0 * NT:t1 * NT])
                for t in range(max(t0, 1), t1):
                    # previous alphas: re-centered copy on RC boundaries,
                    # else the raw ahist slice (argmax is shift-invariant)
                    prev = rel16[:] if (t - 1) % RC == 0 \
                        else ahist[:, (t - 1) * AST:(t - 1) * AST + NT]
                    # i-major scores: s16[p, i*50+j] = prev[i] + T[i, j]
                    nc.vector.tensor_tensor(
                        s16[:].rearrange('p (i j) -> p i j', i=NT),
                        trep[:].rearrange('p (i j) -> p i j', i=NT),
                        prev.unsqueeze(2).broadcast_to([BL, NT, NT]),
                        op=_AluOp.add)
                    # all-tensor_tensor max tree over contiguous i-halves:
                    # every level dual-pumps (2 elem/cyc) vs reduce's 1/cyc
                    nc.vector.tensor_tensor(
                        h16[:], s16[:, 0:1250], s16[:, 1250:2500],
                        op=_AluOp.max)                      # 25 rows
                    nc.vector.tensor_tensor(
                        g16[:], h16[:, 0:600], h16[:, 600:1200],
                        op=_AluOp.max)                      # 12 rows (+carry 24)
                    nc.vector.tensor_tensor(
                        s16[:, 0:300], g16[:, 0:300], g16[:, 300:600],
                        op=_AluOp.max)                      # 6 rows
                    nc.vector.tensor_tensor(
                        s16[:, 400:550], s16[:, 0:150], s16[:, 150:300],
                        op=_AluOp.max)                      # 3 rows
                    red = tmp.tile([BL, NT], F16, tag='red')
                    nc.vector.tensor_tensor(
                        red[:], s16[:, 400:450], s16[:, 450:500],
                        op=_AluOp.max)
                    nc.vector.tensor_tensor(
                        red[:], red[:], s16[:, 500:550], op=_AluOp.max)
                    nc.vector.tensor_tensor(
                        red[:], red[:], h16[:, 1200:1250], op=_AluOp.max)
                    # ahist_t = red + f_t (fp16)
                    nc.vector.tensor_tensor(
                        ahist[:, t * AST:t * AST + NT], red[:],
                        ft[:, (t - t0) * NT:(t - t0 + 1) * NT], op=_AluOp.add)
                    if t % RC == 0:
                        nc.vector.reduce_max(
                            dm[:], ahist[:, t * AST:t * AST + NT], axis=_Axis.X)
                        nc.vector.tensor_scalar(
                            rel16[:], in0=ahist[:, t * AST:t * AST + NT],
                            scalar1=dm[:], scalar2=None, op0=_AluOp.subtract)

            fwd_scope.__exit__(None, None, None)
            cb_scope = nc.named_scope('cbpre')
            cb_scope.__enter__()
            # ---- best-last candidates (fp16), in 8-step micro-chunks ----
            # Emitted lazily: each chunk becomes 6 small DVE ops that slot
            # into the traceback's PE-leg idle windows (one op per tb step)
            # instead of serializing with the DVE-saturated forward phase.
            CBC = 8

            def _cb_chunk_ops(t0):
                tc_n = min(CBC, s_len - t0)
                av = ahist[:, t0 * AST:(t0 + tc_n) * AST].rearrange(
                    'p (t i) -> p t i', t=tc_n)[:, :, 0:NT]
                cs = cbpool.tile([BL, CBC * NT], F16, tag='cs')
                csv = cs[:, 0:tc_n * NT].rearrange('p (t i) -> p t i', t=tc_n)
                q = cbpool.tile([BL, CBC * NT], F16, tag='q')
                qv = q[:, 0:tc_n * NT].rearrange('p (t i) -> p t i', t=tc_n)
                yield lambda: nc.vector.tensor_tensor(
                    csv, av, tstop[:].unsqueeze(1).broadcast_to([BL, tc_n, NT]),
                    op=_AluOp.add)
                yield lambda: nc.vector.reduce_max(
                    mall16[:, t0:t0 + tc_n], csv, axis=_Axis.X)
                yield lambda: nc.vector.tensor_tensor(
                    qv, csv,
                    mall16[:, t0:t0 + tc_n].unsqueeze(2).broadcast_to(
                        [BL, tc_n, NT]),
                    op=_AluOp.is_equal)
                yield lambda: nc.vector.tensor_tensor(
                    csv, qv, iota16[:].unsqueeze(1).broadcast_to([BL, tc_n, NT]),
                    op=_AluOp.mult)
                yield lambda: nc.vector.tensor_reduce(
                    cball16[:, t0:t0 + tc_n], csv, axis=_Axis.X, op=_AluOp.min)
                yield lambda: nc.vector.tensor_scalar(
                    cballu[:, t0:t0 + tc_n], in0=cball16[:, t0:t0 + tc_n],
                    scalar1=BIGF, scalar2=None, op0=_AluOp.add)

            chunk_starts = [t0 for t0 in range(0, s_len, CBC)
                            if t0 + min(CBC, s_len - t0) > tmin]
            chunk_starts.sort(reverse=True)        # tb consumes high t first
            # chunks for the top 16 timesteps must complete before tb starts
            n_upfront = 0
            while n_upfront < len(chunk_starts) and \
                    chunk_starts[n_upfront] + CBC > s_len - 16:
                n_upfront += 1
            for t0 in chunk_starts[:n_upfront]:
                for op in _cb_chunk_ops(t0):
                    op()
            cb_stream = []
            for t0 in chunk_starts[n_upfront:]:
                cb_stream.extend(_cb_chunk_ops(t0))
            cb_stream.reverse()                    # pop() yields in order

            cb_scope.__exit__(None, None, None)
            tb_scope = nc.named_scope('tb')
            tb_scope.__enter__()
            # ---- traceback: decall8[:, 8t] holds the u32 ptr/tag ----
            # min sequence length is S//4, so no reset fires below tmin
            for t in range(s_len - 1, -1, -1):
                if t >= tmin:
                    nc.vector.copy_predicated(decall8[:, 8 * t:8 * t + 1],
                                              eqt8[:, t:t + 1],
                                              cballu[:, t:t + 1])
                if t == 0:
                    break
                # one-hot of current pointer -> PE transpose -> one bf16
                # matmul gathers tcol = T_bf16[:, ptr]
                oh = tmp.tile([BL, NT], BF16, tag='oh')
                nc.vector.tensor_tensor(
                    oh[:], iota[:],
                    decall8[:, 8 * t:8 * t + 1].broadcast_to([BL, NT]),
                    op=_AluOp.is_equal)
                ohT_ps = psum.tile([NT, BL], BF16, tag='ohT')
                nc.tensor.transpose(ohT_ps[:], oh[:], ident[:])
                ohT = tmp.tile([NT, BL], BF16, tag='ohTs')
                nc.vector.tensor_copy(ohT[:], ohT_ps[:])
                tcol_ps = psum.tile([BL, NT], F32, tag='tcol')
                nc.tensor.matmul(tcol_ps[:], lhsT=ohT[:], rhs=tbf[:],
                                 start=True, stop=True)
                # s = ahist_{t-1} + tcol; argmax via max8 + max_index
                s = tmp.tile([BL, NT], F32, tag='s')
                nc.vector.tensor_tensor(
                    s[:], ahist[:, (t - 1) * AST:(t - 1) * AST + NT], tcol_ps[:],
                    op=_AluOp.add)
                m8 = tmp.tile([BL, 8], F32, tag='m8')
                nc.vector.max(m8[:], s[:])
                nc.vector.max_index(decall8[:, 8 * (t - 1):8 * t], m8[:], s[:])
                if cb_stream and t <= s_len - 2:
                    cb_stream.pop()()

            tb_scope.__exit__(None, None, None)
            # decoded tag = decall8 slot0 * active_mask, as int32
            decf = res.tile([BL, s_len], F32, tag='decf')
            dview = decall8[:].rearrange('p (t e) -> p t e', e=8)[:, :, 0:1] \
                .rearrange('p t e -> p (t e)')
            nc.vector.tensor_tensor(decf[:], dview, actf[:], op=_AluOp.mult)
            deci = res.tile([BL, s_len], I32, tag='deci')
            nc.vector.tensor_copy(deci[:], decf[:])
            nc.gpsimd.dma_start(dec_d[:], deci[:])
            if _DEBUG_DUMP:
                nc.gpsimd.dma_start(dbga_d[:], ahist[:, 0:s_len * NT])
                nc.gpsimd.dma_start(dbgc_d[:], cballu[:])

    _split_waits(nc)
    return nc


_CACHE = {}


def _get_program(s_len, tmin):
    key = (s_len, tmin)
    if key not in _CACHE:
        _CACHE[key] = _build_program(s_len, tmin)
    return _CACHE[key]


def kernel(feats, mask, tags, transitions, _trace=False):
    del tags  # unused by Viterbi decode
    feats = np.asarray(feats, dtype=np.float32)
    mask = np.asarray(mask)
    transitions = np.asarray(transitions, dtype=np.float32)
    b, s, tfull = feats.shape
    assert (b, tfull) == (B, TFULL)

    lengths = np.maximum(mask.astype(bool).sum(axis=1), 1).astype(np.int64)  # [B]
    lenm1 = (lengths - 1)[:, None]                                            # [B,1]
    trange = np.arange(s)[None, :]
    eqt8 = (trange == lenm1).astype(np.int8)
    actf = (trange <= lenm1).astype(np.float32)

    import ml_dtypes
    fr = feats[:, :, :NT]                                    # real-tag emissions
    alpha0 = transitions[START, :NT][None, :] + fr[:, 0, :]  # [B, NT] f32
    rel0 = (alpha0 - alpha0.max(axis=1, keepdims=True)).astype(np.float16)
    ftime = np.ascontiguousarray(fr, dtype=np.float16).reshape(B, s * NT)

    t16 = np.ascontiguousarray(
        transitions[:NT, :NT].astype(np.float16))            # [i,j] fp16
    trep = np.ascontiguousarray(
        np.broadcast_to(t16.reshape(1, NT * NT), (BL, NT * NT)))
    tstop = np.ascontiguousarray(np.broadcast_to(
        transitions[:NT, STOP].astype(np.float16)[None, :], (BL, NT)))
    iotau = np.ascontiguousarray(np.broadcast_to(
        np.arange(NT, dtype=np.uint32)[None, :], (BL, NT)))
    iotamb16 = np.ascontiguousarray(np.broadcast_to(
        (np.arange(NT, dtype=np.float16) - np.float16(BIGF))[None, :],
        (BL, NT)))
    ident = np.eye(BL, dtype=ml_dtypes.bfloat16)
    tbf = np.ascontiguousarray(
        transitions[:NT, :NT].T.astype(ml_dtypes.bfloat16))  # [j,i]: row c = T[:,c]

    tmin = max(0, int(lengths.min()) - 1)
    nc = _get_program(s, tmin)
    in_maps = []
    for c in range(NCORES):
        sl = slice(c * BL, (c + 1) * BL)
        in_maps.append({
            'ftime': ftime[sl], 'rel0': np.ascontiguousarray(rel0[sl]),
            'eqt8': np.ascontiguousarray(eqt8[sl]),
            'actf': np.ascontiguousarray(actf[sl]),
            'trep': trep, 'tstop': tstop, 'iotau': iotau,
            'iotamb16': iotamb16, 'ident': ident, 'tbf': tbf,
        })
    res = run_bass_kernel_spmd(nc, in_maps, list(range(NCORES)), trace=_trace)
    out = np.concatenate([res.results[c]['dec'] for c in range(NCORES)], axis=0)
    if _trace:
        kernel._last_results = res
    return out.astype(np.int32)


# revision 39
# speedup vs baseline: 1.2409x; 1.2409x over previous
"""CRF Viterbi decode kernel for Trainium2 (8 NeuronCores, data-parallel).

Problem: B=1024, S=512, TAGSET=50 (T=52 incl START/STOP).
Strategy (fp16 relative-alpha forward + slim u32 traceback):
  - Shard batch across 8 cores (128 batches/core = 128 SBUF partitions).
  - Forward pass entirely in fp16 on the DVE (2x dual-pump throughput for
    tensor_tensor vs fp32): alpha is kept RELATIVE to its per-batch running
    max, re-centered every RC=8 steps, so values stay in fp16's precise
    range (~[-9, 22], quantization ~1e-2 worst case).  The decode argmax is
    invariant to per-(batch,t) constant shifts; measured decode rel-err
    2.9e-3 against the exact reference (budget 2e-2).
      s16[j,i] = prev[i] + T16[j,i]      fp16 tensor_tensor, 2500/partition
      h16      = max(s16[...,:25], s16[...,25:])   (reduce runs 1 elem/cyc,
      red16[j] = max_i h16[j,i]                     tt-max runs 2/cyc, so
      ahist_t  = red16 + f16_t                      pre-combining halves it)
      every RC steps: dm = max_j ahist_t; prev = ahist_t - dm
    ahist rows are stored with a 52-element stride (4B-aligned slices).
  - Best-last candidates per t vectorized in fp16 (first-index argmax via
    the (iota-1024)-min trick); chunks below the minimum length are skipped.
    The scheduler interleaves this into forward-pass DMA bubbles.
  - Traceback: sequential pointer chase, one u32 ptr slot group per step
    (decall8[:, 8t]): one-hot = tensor_tensor is_equal(iota_u32, ptr
    broadcast) -> PE transpose -> one bf16 matmul gathers T[:, ptr] (bf16
    error ~2e-3 is within the noise floor) -> s = ahist[t-1] + tcol ->
    MAX8 + FIND_INDEX8 write the argmax ptr directly into the next step's
    slot.  The baseline's f-value gather is dropped (adding a per-column
    constant cannot change the argmax), and the length-reset predicated
    copy is skipped below the minimum sequence length.
"""
import sys
import types

import numpy as np

import concourse.bass as bass
import concourse.tile as tile
from concourse import mybir
from concourse.bass_utils import run_bass_kernel_spmd


def _ensure_ntff_hook():
    """The agent image's antenv lacks axon_hooks; shim it so trace=True can
    collect NTFF profiles via the ctypes hook in trn_agent_boot."""
    try:
        from antenv.axon_hooks import get_axon_ntff_profile_hook  # noqa: F401
        return
    except ImportError:
        pass
    try:
        import trn_agent_boot.trn_boot as tb
        mod = types.ModuleType('antenv.axon_hooks')
        _h = [None]
        mod.set_axon_ntff_profile_hook = lambda h: _h.__setitem__(0, h)
        mod.get_axon_ntff_profile_hook = lambda: _h[0]
        sys.modules['antenv.axon_hooks'] = mod
        mod.set_axon_ntff_profile_hook(
            tb._ntff_profile_via_ctypes('/opt/axon/libaxon_pjrt.so'))
    except Exception:
        pass


_ensure_ntff_hook()

F32 = mybir.dt.float32
F16 = mybir.dt.float16
BF16 = mybir.dt.bfloat16
I32 = mybir.dt.int32
I8 = mybir.dt.int8

# Problem constants (hardcoded per the harness contract).
B, S, TFULL = 1024, 512, 52
NT = 50                     # real tags; START/STOP can never win (margin ~1e4)
START, STOP = 50, 51
NCORES = 8
BL = B // NCORES            # 128 batches per core = 128 partitions
BIGF = 1024.0               # iota offset for first-index argmin trick (fp16-exact)
FCH = 64 # feats DMA chunk (timesteps per DMA)

_AluOp = mybir.AluOpType
_Axis = mybir.AxisListType

_SPLICE_N = [0]
_DEBUG_DUMP = False


def _split_waits(nc, max_waits=1):
    """This walrus build encodes at most one sync wait per instruction; hoist
    extra waits onto injected same-engine NoOps (engine queues are in-order,
    so semantics are preserved)."""
    for f in nc.m.functions:
        for b in f.blocks:
            insts = b.instructions
            i = 0
            while i < len(insts):
                inst = insts[i]
                si = inst.sync_info
                waits = list(si.on_wait) if si is not None and si.on_wait else []
                if len(waits) > max_waits:
                    si.on_wait = waits[-max_waits:]
                    for w in waits[:-max_waits]:
                        _SPLICE_N[0] += 1
                        nop = mybir.InstNoOp(name=f"I-wsplit{_SPLICE_N[0]}")
                        nop.engine = inst.engine
                        nop.sync_info = mybir.SyncInfo(on_wait=[w], on_update=[])
                        insts.insert(i, nop)
                        i += 1
                i += 1


def _build_program(s_len, tmin):
    """Build the per-core Bass program. Identical on all cores (SPMD)."""
    nc = bass.Bass('TRN2', target_bir_lowering=False, debug=False)

    ftime_d = nc.dram_tensor('ftime', [BL, s_len * NT], F16, kind='ExternalInput').ap()
    rel0_d = nc.dram_tensor('rel0', [BL, NT], F16, kind='ExternalInput').ap()
    eqt8_d = nc.dram_tensor('eqt8', [BL, s_len], I8, kind='ExternalInput').ap()
    actf_d = nc.dram_tensor('actf', [BL, s_len], F32, kind='ExternalInput').ap()
    trep_d = nc.dram_tensor('trep', [BL, NT * NT], F16, kind='ExternalInput').ap()
    tstop_d = nc.dram_tensor('tstop', [BL, NT], F16, kind='ExternalInput').ap()
    iota_d = nc.dram_tensor('iotau', [BL, NT], mybir.dt.uint32,
                            kind='ExternalInput').ap()
    iota16_d = nc.dram_tensor('iotamb16', [BL, NT], F16, kind='ExternalInput').ap()
    ident_d = nc.dram_tensor('ident', [BL, BL], BF16, kind='ExternalInput').ap()
    tbf_d = nc.dram_tensor('tbf', [NT, NT], BF16, kind='ExternalInput').ap()
    dec_d = nc.dram_tensor('dec', [BL, s_len], I32, kind='ExternalOutput').ap()
    dbga_d = nc.dram_tensor('dbga', [BL, s_len * NT], F16,
                            kind='ExternalOutput').ap() if _DEBUG_DUMP else None
    dbgc_d = nc.dram_tensor('dbgc', [BL, s_len], F32,
                            kind='ExternalOutput').ap() if _DEBUG_DUMP else None

    with tile.TileContext(nc) as tc:
        with tc.tile_pool(name='res', bufs=1) as res, \
             tc.tile_pool(name='fch', bufs=4) as fpool, \
             tc.tile_pool(name='cbtmp', bufs=2) as cbpool, \
             tc.tile_pool(name='tmp', bufs=3) as tmp, \
             tc.tile_pool(name='ps', bufs=4, space='PSUM') as psum:

            # ---- resident constants & state ----
            trep = res.tile([BL, NT * NT], F16, tag='trep')
            nc.gpsimd.dma_start(trep[:], trep_d[:])
            tstop = res.tile([BL, NT], F16, tag='tstop')
            nc.gpsimd.dma_start(tstop[:], tstop_d[:])
            iota = res.tile([BL, NT], mybir.dt.uint32, tag='iota')
            nc.gpsimd.dma_start(iota[:], iota_d[:])
            iota16 = res.tile([BL, NT], F16, tag='iota16')
            nc.gpsimd.dma_start(iota16[:], iota16_d[:])
            ident = res.tile([BL, BL], BF16, tag='ident')
            nc.gpsimd.dma_start(ident[:], ident_d[:])
            tbf = res.tile([NT, NT], BF16, tag='tbf')
            nc.gpsimd.dma_start(tbf[:], tbf_d[:])
            eqt8 = res.tile([BL, s_len], I8, tag='eqt8')
            nc.gpsimd.dma_start(eqt8[:], eqt8_d[:])
            actf = res.tile([BL, s_len], F32, tag='actf')
            nc.gpsimd.dma_start(actf[:], actf_d[:])

            AST = 52                       # padded step stride (4B-aligned)
            ahist = res.tile([BL, s_len * AST], F16, tag='ahist')
            nc.gpsimd.dma_start(ahist[:, 0:NT], rel0_d[:])

            s16 = res.tile([BL, NT * NT], F16, tag='s16')
            h16 = res.tile([BL, NT * 25], F16, tag='h16')
            rel16 = res.tile([BL, NT], F16, tag='rel16')
            nc.vector.tensor_copy(rel16[:], ahist[:, 0:NT])
            dm = res.tile([BL, 1], F32, tag='dm')
            decall8 = res.tile([BL, s_len * 8], mybir.dt.uint32,
                               tag='decall8')
            nc.vector.memset(decall8[:], 0.0)
            cballu = res.tile([BL, s_len], mybir.dt.uint32, tag='cballu')
            cball16 = res.tile([BL, s_len], F16, tag='cball16')
            mall16 = res.tile([BL, s_len], F16, tag='mall16')

            # ---- forward (all fp16 on DVE; re-center every RC steps) ----
            RC = 8
            fwd_scope = nc.named_scope('fwd')
            fwd_scope.__enter__()
            n_ch = (s_len + FCH - 1) // FCH
            for c in range(n_ch):
                t0 = c * FCH
                t1 = min(t0 + FCH, s_len)
                ft = fpool.tile([BL, (t1 - t0) * NT], F16, tag='fch')
                nc.gpsimd.dma_start(ft[:], ftime_d[:, t0 * NT:t1 * NT])
                for t in range(max(t0, 1), t1):
                    # previous alphas: re-centered copy on RC boundaries,
                    # else the raw ahist slice (argmax is shift-invariant)
                    prev = rel16[:] if (t - 1) % RC == 0 \
                        else ahist[:, (t - 1) * AST:(t - 1) * AST + NT]
                    nc.vector.tensor_tensor(
                        s16[:].rearrange('p (j i) -> p j i', j=NT),
                        trep[:].rearrange('p (j i) -> p j i', j=NT),
                        prev.unsqueeze(1).broadcast_to([BL, NT, NT]),
                        op=_AluOp.add)
                    # split-combine: tt-max runs at 2 elem/cycle (fp16 dual
                    # pump) while reduce is 1/cycle; halve the reduce's input
                    s3 = s16[:].rearrange('p (j i) -> p j i', j=NT)
                    nc.vector.tensor_tensor(
                        h16[:].rearrange('p (j i) -> p j i', j=NT),
                        s3[:, :, 0:25], s3[:, :, 25:50], op=_AluOp.max)
                    red = tmp.tile([BL, NT], F16, tag='red')
                    nc.vector.reduce_max(
                        red[:], h16[:].rearrange('p (j i) -> p j i', j=NT),
                        axis=_Axis.X)
                    # ahist_t = red + f_t (fp16)
                    nc.vector.tensor_tensor(
                        ahist[:, t * AST:t * AST + NT], red[:],
                        ft[:, (t - t0) * NT:(t - t0 + 1) * NT], op=_AluOp.add)
                    if t % RC == 0:
                        nc.vector.reduce_max(
                            dm[:], ahist[:, t * AST:t * AST + NT], axis=_Axis.X)
                        nc.vector.tensor_scalar(
                            rel16[:], in0=ahist[:, t * AST:t * AST + NT],
                            scalar1=dm[:], scalar2=None, op0=_AluOp.subtract)

            fwd_scope.__exit__(None, None, None)
            cb_scope = nc.named_scope('cbpre')
            cb_scope.__enter__()
            # ---- best-last candidates (fp16), in 8-step micro-chunks ----
            # Emitted lazily: each chunk becomes 6 small DVE ops that slot
            # into the traceback's PE-leg idle windows (one op per tb step)
            # instead of serializing with the DVE-saturated forward phase.
            CBC = 8

            def _cb_chunk_ops(t0):
                tc_n = min(CBC, s_len - t0)
                av = ahist[:, t0 * AST:(t0 + tc_n) * AST].rearrange(
                    'p (t i) -> p t i', t=tc_n)[:, :, 0:NT]
                cs = cbpool.tile([BL, CBC * NT], F16, tag='cs')
                csv = cs[:, 0:tc_n * NT].rearrange('p (t i) -> p t i', t=tc_n)
                q = cbpool.tile([BL, CBC * NT], F16, tag='q')
                qv = q[:, 0:tc_n * NT].rearrange('p (t i) -> p t i', t=tc_n)
                yield lambda: nc.vector.tensor_tensor(
                    csv, av, tstop[:].unsqueeze(1).broadcast_to([BL, tc_n, NT]),
                    op=_AluOp.add)
                yield lambda: nc.vector.reduce_max(
                    mall16[:, t0:t0 + tc_n], csv, axis=_Axis.X)
                yield lambda: nc.vector.tensor_tensor(
                    qv, csv,
                    mall16[:, t0:t0 + tc_n].unsqueeze(2).broadcast_to(
                        [BL, tc_n, NT]),
                    op=_AluOp.is_equal)
                yield lambda: nc.vector.tensor_tensor(
                    csv, qv, iota16[:].unsqueeze(1).broadcast_to([BL, tc_n, NT]),
                    op=_AluOp.mult)
                yield lambda: nc.vector.tensor_reduce(
                    cball16[:, t0:t0 + tc_n], csv, axis=_Axis.X, op=_AluOp.min)
                yield lambda: nc.vector.tensor_scalar(
                    cballu[:, t0:t0 + tc_n], in0=cball16[:, t0:t0 + tc_n],
                    scalar1=BIGF, scalar2=None, op0=_AluOp.add)

            chunk_starts = [t0 for t0 in range(0, s_len, CBC)
                            if t0 + min(CBC, s_len - t0) > tmin]
            chunk_starts.sort(reverse=True)        # tb consumes high t first
            # chunks for the top 16 timesteps must complete before tb starts
            n_upfront = 0
            while n_upfront < len(chunk_starts) and \
                    chunk_starts[n_upfront] + CBC > s_len - 16:
                n_upfront += 1
            for t0 in chunk_starts[:n_upfront]:
                for op in _cb_chunk_ops(t0):
                    op()
            cb_stream = []
            for t0 in chunk_starts[n_upfront:]:
                cb_stream.extend(_cb_chunk_ops(t0))
            cb_stream.reverse()                    # pop() yields in order

            cb_scope.__exit__(None, None, None)
            tb_scope = nc.named_scope('tb')
            tb_scope.__enter__()
            # ---- traceback: decall8[:, 8t] holds the u32 ptr/tag ----
            # min sequence length is S//4, so no reset fires below tmin
            for t in range(s_len - 1, -1, -1):
                if t >= tmin:
                    nc.vector.copy_predicated(decall8[:, 8 * t:8 * t + 1],
                                              eqt8[:, t:t + 1],
                                              cballu[:, t:t + 1])
                if t == 0:
                    break
                # one-hot of current pointer -> PE transpose -> one bf16
                # matmul gathers tcol = T_bf16[:, ptr]
                oh = tmp.tile([BL, NT], BF16, tag='oh')
                nc.vector.tensor_tensor(
                    oh[:], iota[:],
                    decall8[:, 8 * t:8 * t + 1].broadcast_to([BL, NT]),
                    op=_AluOp.is_equal)
                ohT_ps = psum.tile([NT, BL], BF16, tag='ohT')
                nc.tensor.transpose(ohT_ps[:], oh[:], ident[:])
                ohT = tmp.tile([NT, BL], BF16, tag='ohTs')
                nc.vector.tensor_copy(ohT[:], ohT_ps[:])
                tcol_ps = psum.tile([BL, NT], F32, tag='tcol')
                nc.tensor.matmul(tcol_ps[:], lhsT=ohT[:], rhs=tbf[:],
                                 start=True, stop=True)
                # s = ahist_{t-1} + tcol; argmax via max8 + max_index
                s = tmp.tile([BL, NT], F32, tag='s')
                nc.vector.tensor_tensor(
                    s[:], ahist[:, (t - 1) * AST:(t - 1) * AST + NT], tcol_ps[:],
                    op=_AluOp.add)
                m8 = tmp.tile([BL, 8], F32, tag='m8')
                nc.vector.max(m8[:], s[:])
                nc.vector.max_index(decall8[:, 8 * (t - 1):8 * t], m8[:], s[:])
                if cb_stream and t <= s_len - 2:
                    cb_stream.pop()()

            tb_scope.__exit__(None, None, None)
            # decoded tag = decall8 slot0 * active_mask, as int32
            decf = res.tile([BL, s_len], F32, tag='decf')
            dview = decall8[:].rearrange('p (t e) -> p t e', e=8)[:, :, 0:1] \
                .rearrange('p t e -> p (t e)')
            nc.vector.tensor_tensor(decf[:], dview, actf[:], op=_AluOp.mult)
            deci = res.tile([BL, s_len], I32, tag='deci')
            nc.vector.tensor_copy(deci[:], decf[:])
            nc.gpsimd.dma_start(dec_d[:], deci[:])
            if _DEBUG_DUMP:
                nc.gpsimd.dma_start(dbga_d[:], ahist[:, 0:s_len * NT])
                nc.gpsimd.dma_start(dbgc_d[:], cballu[:])

    _split_waits(nc)
    return nc


_CACHE = {}


def _get_program(s_len, tmin):
    key = (s_len, tmin)
    if key not in _CACHE:
        _CACHE[key] = _build_program(s_len, tmin)
    return _CACHE[key]


def kernel(feats, mask, tags, transitions, _trace=False):
    del tags  # unused by Viterbi decode
    feats = np.asarray(feats, dtype=np.float32)
    mask = np.asarray(mask)
    transitions = np.asarray(transitions, dtype=np.float32)
    b, s, tfull = feats.shape
    assert (b, tfull) == (B, TFULL)

    lengths = np.maximum(mask.astype(bool).sum(axis=1), 1).astype(np.int64)  # [B]
    lenm1 = (lengths - 1)[:, None]                                            # [B,1]
    trange = np.arange(s)[None, :]
    eqt8 = (trange == lenm1).astype(np.int8)
    actf = (trange <= lenm1).astype(np.float32)

    import ml_dtypes
    fr = feats[:, :, :NT]                                    # real-tag emissions
    alpha0 = transitions[START, :NT][None, :] + fr[:, 0, :]  # [B, NT] f32
    rel0 = (alpha0 - alpha0.max(axis=1, keepdims=True)).astype(np.float16)
    ftime = np.ascontiguousarray(fr, dtype=np.float16).reshape(B, s * NT)

    transT16 = np.ascontiguousarray(
        transitions[:NT, :NT].T.astype(np.float16))          # [j,i] fp16
    trep = np.ascontiguousarray(
        np.broadcast_to(transT16.reshape(1, NT * NT), (BL, NT * NT)))
    tstop = np.ascontiguousarray(np.broadcast_to(
        transitions[:NT, STOP].astype(np.float16)[None, :], (BL, NT)))
    iotau = np.ascontiguousarray(np.broadcast_to(
        np.arange(NT, dtype=np.uint32)[None, :], (BL, NT)))
    iotamb16 = np.ascontiguousarray(np.broadcast_to(
        (np.arange(NT, dtype=np.float16) - np.float16(BIGF))[None, :],
        (BL, NT)))
    ident = np.eye(BL, dtype=ml_dtypes.bfloat16)
    tbf = np.ascontiguousarray(
        transitions[:NT, :NT].T.astype(ml_dtypes.bfloat16))  # [j,i]: row c = T[:,c]

    tmin = max(0, int(lengths.min()) - 1)
    nc = _get_program(s, tmin)
    in_maps = []
    for c in range(NCORES):
        sl = slice(c * BL, (c + 1) * BL)
        in_maps.append({
            'ftime': ftime[sl], 'rel0': np.ascontiguousarray(rel0[sl]),
            'eqt8': np.ascontiguousarray(eqt8[sl]),
            'actf': np.ascontiguousarray(actf[sl]),
            'trep': trep, 'tstop': tstop, 'iotau': iotau,
            'iotamb16': iotamb16, 'ident': ident, 'tbf': tbf,
        })
    res = run_bass_kernel_spmd(nc, in_maps, list(range(NCORES)), trace=_trace)
    out = np.concatenate([res.results[c]['dec'] for c in range(NCORES)], axis=0)
    if _trace:
        kernel._last_results = res
    return out.astype(np.int32)


# revision 40
# speedup vs baseline: 1.2410x; 1.0001x over previous
"""CRF Viterbi decode kernel for Trainium2 (8 NeuronCores, data-parallel).

Problem: B=1024, S=512, TAGSET=50 (T=52 incl START/STOP).
Strategy (fp16 relative-alpha forward + slim u32 traceback):
  - Shard batch across 8 cores (128 batches/core = 128 SBUF partitions).
  - Forward pass entirely in fp16 on the DVE (2x dual-pump throughput for
    tensor_tensor vs fp32): alpha is kept RELATIVE to its per-batch running
    max, re-centered every RC=8 steps, so values stay in fp16's precise
    range (~[-9, 22], quantization ~1e-2 worst case).  The decode argmax is
    invariant to per-(batch,t) constant shifts; measured decode rel-err
    2.9e-3 against the exact reference (budget 2e-2).
      s16[j,i] = prev[i] + T16[j,i]      fp16 tensor_tensor, 2500/partition
      h16      = max(s16[...,:25], s16[...,25:])   (reduce runs 1 elem/cyc,
      red16[j] = max_i h16[j,i]                     tt-max runs 2/cyc, so
      ahist_t  = red16 + f16_t                      pre-combining halves it)
      every RC steps: dm = max_j ahist_t; prev = ahist_t - dm
    ahist rows are stored with a 52-element stride (4B-aligned slices).
  - Best-last candidates per t vectorized in fp16 (first-index argmax via
    the (iota-1024)-min trick); chunks below the minimum length are skipped.
    Emitted as 8-step micro-chunks dripped one op per traceback step so they
    fill the traceback's PE-leg idle windows instead of serializing with the
    DVE-saturated forward phase.
  - Traceback: sequential pointer chase, one u32 ptr slot group per step
    (decall8[:, 8t]): one-hot = tensor_tensor is_equal(iota_u32, ptr
    broadcast) -> PE transpose -> one bf16 matmul gathers T[:, ptr] (bf16
    error ~2e-3 is within the noise floor) -> s = ahist[t-1] + tcol ->
    MAX8 + FIND_INDEX8 write the argmax ptr directly into the next step's
    slot.  The baseline's f-value gather is dropped (adding a per-column
    constant cannot change the argmax), and the length-reset predicated
    copy is skipped below the minimum sequence length.
"""
import sys
import types

import numpy as np

import concourse.bass as bass
import concourse.tile as tile
from concourse import mybir
from concourse.bass_utils import run_bass_kernel_spmd


def _ensure_ntff_hook():
    """The agent image's antenv lacks axon_hooks; shim it so trace=True can
    collect NTFF profiles via the ctypes hook in trn_agent_boot."""
    try:
        from antenv.axon_hooks import get_axon_ntff_profile_hook  # noqa: F401
        return
    except ImportError:
        pass
    try:
        import trn_agent_boot.trn_boot as tb
        mod = types.ModuleType('antenv.axon_hooks')
        _h = [None]
        mod.set_axon_ntff_profile_hook = lambda h: _h.__setitem__(0, h)
        mod.get_axon_ntff_profile_hook = lambda: _h[0]
        sys.modules['antenv.axon_hooks'] = mod
        mod.set_axon_ntff_profile_hook(
            tb._ntff_profile_via_ctypes('/opt/axon/libaxon_pjrt.so'))
    except Exception:
        pass


_ensure_ntff_hook()

F32 = mybir.dt.float32
F16 = mybir.dt.float16
BF16 = mybir.dt.bfloat16
I32 = mybir.dt.int32
I8 = mybir.dt.int8

# Problem constants (hardcoded per the harness contract).
B, S, TFULL = 1024, 512, 52
NT = 50                     # real tags; START/STOP can never win (margin ~1e4)
START, STOP = 50, 51
NCORES = 8
BL = B // NCORES            # 128 batches per core = 128 partitions
BIGF = 1024.0               # iota offset for first-index argmin trick (fp16-exact)
FCH = 64 # feats DMA chunk (timesteps per DMA)

_AluOp = mybir.AluOpType
_Axis = mybir.AxisListType

_SPLICE_N = [0]
_DEBUG_DUMP = False


def _split_waits(nc, max_waits=1):
    """This walrus build encodes at most one sync wait per instruction; hoist
    extra waits onto injected same-engine NoOps (engine queues are in-order,
    so semantics are preserved)."""
    for f in nc.m.functions:
        for b in f.blocks:
            insts = b.instructions
            i = 0
            while i < len(insts):
                inst = insts[i]
                si = inst.sync_info
                waits = list(si.on_wait) if si is not None and si.on_wait else []
                if len(waits) > max_waits:
                    si.on_wait = waits[-max_waits:]
                    for w in waits[:-max_waits]:
                        _SPLICE_N[0] += 1
                        nop = mybir.InstNoOp(name=f"I-wsplit{_SPLICE_N[0]}")
                        nop.engine = inst.engine
                        nop.sync_info = mybir.SyncInfo(on_wait=[w], on_update=[])
                        insts.insert(i, nop)
                        i += 1
                i += 1


def _build_program(s_len, tmin):
    """Build the per-core Bass program. Identical on all cores (SPMD)."""
    nc = bass.Bass('TRN2', target_bir_lowering=False, debug=False)

    ftime_d = nc.dram_tensor('ftime', [BL, s_len * NT], F16, kind='ExternalInput').ap()
    rel0_d = nc.dram_tensor('rel0', [BL, NT], F16, kind='ExternalInput').ap()
    eqt8_d = nc.dram_tensor('eqt8', [BL, s_len], I8, kind='ExternalInput').ap()
    actf_d = nc.dram_tensor('actf', [BL, s_len], F32, kind='ExternalInput').ap()
    trep_d = nc.dram_tensor('trep', [BL, NT * NT], F16, kind='ExternalInput').ap()
    tstop_d = nc.dram_tensor('tstop', [BL, NT], F16, kind='ExternalInput').ap()
    iota_d = nc.dram_tensor('iotau', [BL, NT], mybir.dt.uint32,
                            kind='ExternalInput').ap()
    iota16_d = nc.dram_tensor('iotamb16', [BL, NT], F16, kind='ExternalInput').ap()
    ident_d = nc.dram_tensor('ident', [BL, BL], BF16, kind='ExternalInput').ap()
    tbf_d = nc.dram_tensor('tbf', [NT, NT], BF16, kind='ExternalInput').ap()
    dec_d = nc.dram_tensor('dec', [BL, s_len], I32, kind='ExternalOutput').ap()
    dbga_d = nc.dram_tensor('dbga', [BL, s_len * NT], F16,
                            kind='ExternalOutput').ap() if _DEBUG_DUMP else None
    dbgc_d = nc.dram_tensor('dbgc', [BL, s_len], F32,
                            kind='ExternalOutput').ap() if _DEBUG_DUMP else None

    with tile.TileContext(nc) as tc:
        with tc.tile_pool(name='res', bufs=1) as res, \
             tc.tile_pool(name='fch', bufs=4) as fpool, \
             tc.tile_pool(name='cbtmp', bufs=2) as cbpool, \
             tc.tile_pool(name='tmp', bufs=3) as tmp, \
             tc.tile_pool(name='ps', bufs=4, space='PSUM') as psum:

            # ---- resident constants & state ----
            trep = res.tile([BL, NT * NT], F16, tag='trep')
            nc.gpsimd.dma_start(trep[:], trep_d[:])
            tstop = res.tile([BL, NT], F16, tag='tstop')
            nc.gpsimd.dma_start(tstop[:], tstop_d[:])
            iota = res.tile([BL, NT], mybir.dt.uint32, tag='iota')
            nc.gpsimd.dma_start(iota[:], iota_d[:])
            iota16 = res.tile([BL, NT], F16, tag='iota16')
            nc.gpsimd.dma_start(iota16[:], iota16_d[:])
            ident = res.tile([BL, BL], BF16, tag='ident')
            nc.gpsimd.dma_start(ident[:], ident_d[:])
            tbf = res.tile([NT, NT], BF16, tag='tbf')
            nc.gpsimd.dma_start(tbf[:], tbf_d[:])
            eqt8 = res.tile([BL, s_len], I8, tag='eqt8')
            nc.gpsimd.dma_start(eqt8[:], eqt8_d[:])
            actf = res.tile([BL, s_len], F32, tag='actf')
            nc.gpsimd.dma_start(actf[:], actf_d[:])

            AST = 52                       # padded step stride (4B-aligned)
            ahist = res.tile([BL, s_len * AST], F16, tag='ahist')
            nc.gpsimd.dma_start(ahist[:, 0:NT], rel0_d[:])

            s16 = res.tile([BL, NT * NT], F16, tag='s16')
            h16 = res.tile([BL, NT * 25], F16, tag='h16')
            rel16 = res.tile([BL, NT], F16, tag='rel16')
            nc.vector.tensor_copy(rel16[:], ahist[:, 0:NT])
            dm = res.tile([BL, 1], F32, tag='dm')
            decall8 = res.tile([BL, s_len * 8], mybir.dt.uint32,
                               tag='decall8')
            nc.vector.memset(decall8[:], 0.0)
            cballu = res.tile([BL, s_len], mybir.dt.uint32, tag='cballu')
            cball16 = res.tile([BL, s_len], F16, tag='cball16')
            mall16 = res.tile([BL, s_len], F16, tag='mall16')

            # ---- forward (all fp16 on DVE; re-center every RC steps) ----
            RC = 8
            fwd_scope = nc.named_scope('fwd')
            fwd_scope.__enter__()
            n_ch = (s_len + FCH - 1) // FCH
            for c in range(n_ch):
                t0 = c * FCH
                t1 = min(t0 + FCH, s_len)
                ft = fpool.tile([BL, (t1 - t0) * NT], F16, tag='fch')
                nc.gpsimd.dma_start(ft[:], ftime_d[:, t0 * NT:t1 * NT])
                for t in range(max(t0, 1), t1):
                    # previous alphas: re-centered copy on RC boundaries,
                    # else the raw ahist slice (argmax is shift-invariant)
                    prev = rel16[:] if (t - 1) % RC == 0 \
                        else ahist[:, (t - 1) * AST:(t - 1) * AST + NT]
                    nc.vector.tensor_tensor(
                        s16[:].rearrange('p (j i) -> p j i', j=NT),
                        trep[:].rearrange('p (j i) -> p j i', j=NT),
                        prev.unsqueeze(1).broadcast_to([BL, NT, NT]),
                        op=_AluOp.add)
                    # split-combine: tt-max runs at 2 elem/cycle (fp16 dual
                    # pump) while reduce is 1/cycle; halve the reduce's input
                    s3 = s16[:].rearrange('p (j i) -> p j i', j=NT)
                    nc.vector.tensor_tensor(
                        h16[:].rearrange('p (j i) -> p j i', j=NT),
                        s3[:, :, 0:25], s3[:, :, 25:50], op=_AluOp.max)
                    red = tmp.tile([BL, NT], F16, tag='red')
                    nc.vector.reduce_max(
                        red[:], h16[:].rearrange('p (j i) -> p j i', j=NT),
                        axis=_Axis.X)
                    # ahist_t = red + f_t (fp16)
                    nc.vector.tensor_tensor(
                        ahist[:, t * AST:t * AST + NT], red[:],
                        ft[:, (t - t0) * NT:(t - t0 + 1) * NT], op=_AluOp.add)
                    if t % RC == 0:
                        nc.vector.reduce_max(
                            dm[:], ahist[:, t * AST:t * AST + NT], axis=_Axis.X)
                        nc.vector.tensor_scalar(
                            rel16[:], in0=ahist[:, t * AST:t * AST + NT],
                            scalar1=dm[:], scalar2=None, op0=_AluOp.subtract)

            fwd_scope.__exit__(None, None, None)
            cb_scope = nc.named_scope('cbpre')
            cb_scope.__enter__()
            # ---- best-last candidates (fp16), in 8-step micro-chunks ----
            # Emitted lazily: each chunk becomes 6 small DVE ops that slot
            # into the traceback's PE-leg idle windows (one op per tb step)
            # instead of serializing with the DVE-saturated forward phase.
            CBC = 8

            def _cb_chunk_ops(t0):
                tc_n = min(CBC, s_len - t0)
                av = ahist[:, t0 * AST:(t0 + tc_n) * AST].rearrange(
                    'p (t i) -> p t i', t=tc_n)[:, :, 0:NT]
                cs = cbpool.tile([BL, CBC * NT], F16, tag='cs')
                csv = cs[:, 0:tc_n * NT].rearrange('p (t i) -> p t i', t=tc_n)
                q = cbpool.tile([BL, CBC * NT], F16, tag='q')
                qv = q[:, 0:tc_n * NT].rearrange('p (t i) -> p t i', t=tc_n)
                yield lambda: nc.vector.tensor_tensor(
                    csv, av, tstop[:].unsqueeze(1).broadcast_to([BL, tc_n, NT]),
                    op=_AluOp.add)
                yield lambda: nc.vector.reduce_max(
                    mall16[:, t0:t0 + tc_n], csv, axis=_Axis.X)
                yield lambda: nc.vector.tensor_tensor(
                    qv, csv,
                    mall16[:, t0:t0 + tc_n].unsqueeze(2).broadcast_to(
                        [BL, tc_n, NT]),
                    op=_AluOp.is_equal)
                yield lambda: nc.vector.tensor_tensor(
                    csv, qv, iota16[:].unsqueeze(1).broadcast_to([BL, tc_n, NT]),
                    op=_AluOp.mult)
                yield lambda: nc.vector.tensor_reduce(
                    cball16[:, t0:t0 + tc_n], csv, axis=_Axis.X, op=_AluOp.min)
                yield lambda: nc.vector.tensor_scalar(
                    cballu[:, t0:t0 + tc_n], in0=cball16[:, t0:t0 + tc_n],
                    scalar1=BIGF, scalar2=None, op0=_AluOp.add)

            chunk_starts = [t0 for t0 in range(0, s_len, CBC)
                            if t0 + min(CBC, s_len - t0) > tmin]
            chunk_starts.sort(reverse=True)        # tb consumes high t first
            # chunks for the top 16 timesteps must complete before tb starts
            n_upfront = 0
            while n_upfront < len(chunk_starts) and \
                    chunk_starts[n_upfront] + CBC > s_len - 16:
                n_upfront += 1
            for t0 in chunk_starts[:n_upfront]:
                for op in _cb_chunk_ops(t0):
                    op()
            cb_stream = []
            for t0 in chunk_starts[n_upfront:]:
                cb_stream.extend(_cb_chunk_ops(t0))
            cb_stream.reverse()                    # pop() yields in order

            cb_scope.__exit__(None, None, None)
            tb_scope = nc.named_scope('tb')
            tb_scope.__enter__()
            # ---- traceback: decall8[:, 8t] holds the u32 ptr/tag ----
            # min sequence length is S//4, so no reset fires below tmin
            for t in range(s_len - 1, -1, -1):
                if t >= tmin:
                    nc.vector.copy_predicated(decall8[:, 8 * t:8 * t + 1],
                                              eqt8[:, t:t + 1],
                                              cballu[:, t:t + 1])
                if t == 0:
                    break
                # one-hot of current pointer -> PE transpose -> one bf16
                # matmul gathers tcol = T_bf16[:, ptr]
                oh = tmp.tile([BL, NT], BF16, tag='oh')
                nc.vector.tensor_tensor(
                    oh[:], iota[:],
                    decall8[:, 8 * t:8 * t + 1].broadcast_to([BL, NT]),
                    op=_AluOp.is_equal)
                ohT_ps = psum.tile([NT, BL], BF16, tag='ohT')
                nc.tensor.transpose(ohT_ps[:], oh[:], ident[:])
                ohT = tmp.tile([NT, BL], BF16, tag='ohTs')
                nc.vector.tensor_copy(ohT[:], ohT_ps[:])
                tcol_ps = psum.tile([BL, NT], F32, tag='tcol')
                nc.tensor.matmul(tcol_ps[:], lhsT=ohT[:], rhs=tbf[:],
                                 start=True, stop=True)
                # s = ahist_{t-1} + tcol; argmax via max8 + max_index
                s = tmp.tile([BL, NT], F32, tag='s')
                nc.vector.tensor_tensor(
                    s[:], ahist[:, (t - 1) * AST:(t - 1) * AST + NT], tcol_ps[:],
                    op=_AluOp.add)
                m8 = tmp.tile([BL, 8], F32, tag='m8')
                nc.vector.max(m8[:], s[:])
                nc.vector.max_index(decall8[:, 8 * (t - 1):8 * t], m8[:], s[:])
                if cb_stream and t <= s_len - 2:
                    cb_stream.pop()()

            tb_scope.__exit__(None, None, None)
            # decoded tag = decall8 slot0 * active_mask, as int32
            decf = res.tile([BL, s_len], F32, tag='decf')
            dview = decall8[:].rearrange('p (t e) -> p t e', e=8)[:, :, 0:1] \
                .rearrange('p t e -> p (t e)')
            nc.vector.tensor_tensor(decf[:], dview, actf[:], op=_AluOp.mult)
            deci = res.tile([BL, s_len], I32, tag='deci')
            nc.vector.tensor_copy(deci[:], decf[:])
            nc.gpsimd.dma_start(dec_d[:], deci[:])
            if _DEBUG_DUMP:
                nc.gpsimd.dma_start(dbga_d[:], ahist[:, 0:s_len * NT])
                nc.gpsimd.dma_start(dbgc_d[:], cballu[:])

    _split_waits(nc)
    return nc


_CACHE = {}


def _get_program(s_len, tmin):
    key = (s_len, tmin)
    if key not in _CACHE:
        _CACHE[key] = _build_program(s_len, tmin)
    return _CACHE[key]


def kernel(feats, mask, tags, transitions, _trace=False):
    del tags  # unused by Viterbi decode
    feats = np.asarray(feats, dtype=np.float32)
    mask = np.asarray(mask)
    transitions = np.asarray(transitions, dtype=np.float32)
    b, s, tfull = feats.shape
    assert (b, tfull) == (B, TFULL)

    lengths = np.maximum(mask.astype(bool).sum(axis=1), 1).astype(np.int64)  # [B]
    lenm1 = (lengths - 1)[:, None]                                            # [B,1]
    trange = np.arange(s)[None, :]
    eqt8 = (trange == lenm1).astype(np.int8)
    actf = (trange <= lenm1).astype(np.float32)

    import ml_dtypes
    fr = feats[:, :, :NT]                                    # real-tag emissions
    alpha0 = transitions[START, :NT][None, :] + fr[:, 0, :]  # [B, NT] f32
    rel0 = (alpha0 - alpha0.max(axis=1, keepdims=True)).astype(np.float16)
    ftime = np.ascontiguousarray(fr, dtype=np.float16).reshape(B, s * NT)

    transT16 = np.ascontiguousarray(
        transitions[:NT, :NT].T.astype(np.float16))          # [j,i] fp16
    trep = np.ascontiguousarray(
        np.broadcast_to(transT16.reshape(1, NT * NT), (BL, NT * NT)))
    tstop = np.ascontiguousarray(np.broadcast_to(
        transitions[:NT, STOP].astype(np.float16)[None, :], (BL, NT)))
    iotau = np.ascontiguousarray(np.broadcast_to(
        np.arange(NT, dtype=np.uint32)[None, :], (BL, NT)))
    iotamb16 = np.ascontiguousarray(np.broadcast_to(
        (np.arange(NT, dtype=np.float16) - np.float16(BIGF))[None, :],
        (BL, NT)))
    ident = np.eye(BL, dtype=ml_dtypes.bfloat16)
    tbf = np.ascontiguousarray(
        transitions[:NT, :NT].T.astype(ml_dtypes.bfloat16))  # [j,i]: row c = T[:,c]

    tmin = max(0, int(lengths.min()) - 1)
    nc = _get_program(s, tmin)
    in_maps = []
    for c in range(NCORES):
        sl = slice(c * BL, (c + 1) * BL)
        in_maps.append({
            'ftime': ftime[sl], 'rel0': np.ascontiguousarray(rel0[sl]),
            'eqt8': np.ascontiguousarray(eqt8[sl]),
            'actf': np.ascontiguousarray(actf[sl]),
            'trep': trep, 'tstop': tstop, 'iotau': iotau,
            'iotamb16': iotamb16, 'ident': ident, 'tbf': tbf,
        })
    res = run_bass_kernel_spmd(nc, in_maps, list(range(NCORES)), trace=_trace)
    out = np.concatenate([res.results[c]['dec'] for c in range(NCORES)], axis=0)
    if _trace:
        kernel._last_results = res
    return out.astype(np.int32)


# revision 41
# speedup vs baseline: 1.2418x; 1.0006x over previous
"""CRF Viterbi decode kernel for Trainium2 (8 NeuronCores, data-parallel).

Problem: B=1024, S=512, TAGSET=50 (T=52 incl START/STOP).
Strategy (fp16 relative-alpha forward + slim u32 traceback):
  - Shard batch across 8 cores (128 batches/core = 128 SBUF partitions).
  - Forward pass entirely in fp16 on the DVE (2x dual-pump throughput for
    tensor_tensor vs fp32): alpha is kept RELATIVE to its per-batch running
    max, re-centered every RC=8 steps, so values stay in fp16's precise
    range (~[-9, 22], quantization ~1e-2 worst case).  The decode argmax is
    invariant to per-(batch,t) constant shifts; measured decode rel-err
    2.9e-3 against the exact reference (budget 2e-2).
      s16[j,i] = prev[i] + T16[j,i]      fp16 tensor_tensor, 2500/partition
      h16      = max(s16[...,:25], s16[...,25:])   (reduce runs 1 elem/cyc,
      red16[j] = max_i h16[j,i]                     tt-max runs 2/cyc, so
      ahist_t  = red16 + f16_t                      pre-combining halves it)
      every RC steps: dm = max_j ahist_t; prev = ahist_t - dm
    ahist rows are stored with a 52-element stride (4B-aligned slices).
  - Best-last candidates per t vectorized in fp16 (first-index argmax via
    the (iota-1024)-min trick); chunks below the minimum length are skipped.
    Emitted as 8-step micro-chunks dripped one op per traceback step so they
    fill the traceback's PE-leg idle windows instead of serializing with the
    DVE-saturated forward phase.
  - Traceback: sequential pointer chase, one u32 ptr slot group per step
    (decall8[:, 8t]): one-hot = tensor_tensor is_equal(iota_u32, ptr
    broadcast) -> PE transpose -> one bf16 matmul gathers T[:, ptr] (bf16
    error ~2e-3 is within the noise floor) -> s = ahist[t-1] + tcol ->
    MAX8 + FIND_INDEX8 write the argmax ptr directly into the next step's
    slot.  The baseline's f-value gather is dropped (adding a per-column
    constant cannot change the argmax), and the length-reset predicated
    copy is skipped below the minimum sequence length.
"""
import sys
import types

import numpy as np

import concourse.bass as bass
import concourse.tile as tile
from concourse import mybir
from concourse.bass_utils import run_bass_kernel_spmd


def _ensure_ntff_hook():
    """The agent image's antenv lacks axon_hooks; shim it so trace=True can
    collect NTFF profiles via the ctypes hook in trn_agent_boot."""
    try:
        from antenv.axon_hooks import get_axon_ntff_profile_hook  # noqa: F401
        return
    except ImportError:
        pass
    try:
        import trn_agent_boot.trn_boot as tb
        mod = types.ModuleType('antenv.axon_hooks')
        _h = [None]
        mod.set_axon_ntff_profile_hook = lambda h: _h.__setitem__(0, h)
        mod.get_axon_ntff_profile_hook = lambda: _h[0]
        sys.modules['antenv.axon_hooks'] = mod
        mod.set_axon_ntff_profile_hook(
            tb._ntff_profile_via_ctypes('/opt/axon/libaxon_pjrt.so'))
    except Exception:
        pass


_ensure_ntff_hook()

F32 = mybir.dt.float32
F16 = mybir.dt.float16
BF16 = mybir.dt.bfloat16
I32 = mybir.dt.int32
I8 = mybir.dt.int8

# Problem constants (hardcoded per the harness contract).
B, S, TFULL = 1024, 512, 52
NT = 50                     # real tags; START/STOP can never win (margin ~1e4)
START, STOP = 50, 51
NCORES = 8
BL = B // NCORES            # 128 batches per core = 128 partitions
BIGF = 1024.0               # iota offset for first-index argmin trick (fp16-exact)
FCH = 64 # feats DMA chunk (timesteps per DMA)

_AluOp = mybir.AluOpType
_Axis = mybir.AxisListType

_SPLICE_N = [0]
_DEBUG_DUMP = False


def _split_waits(nc, max_waits=1):
    """This walrus build encodes at most one sync wait per instruction; hoist
    extra waits onto injected same-engine NoOps (engine queues are in-order,
    so semantics are preserved)."""
    for f in nc.m.functions:
        for b in f.blocks:
            insts = b.instructions
            i = 0
            while i < len(insts):
                inst = insts[i]
                si = inst.sync_info
                waits = list(si.on_wait) if si is not None and si.on_wait else []
                if len(waits) > max_waits:
                    si.on_wait = waits[-max_waits:]
                    for w in waits[:-max_waits]:
                        _SPLICE_N[0] += 1
                        nop = mybir.InstNoOp(name=f"I-wsplit{_SPLICE_N[0]}")
                        nop.engine = inst.engine
                        nop.sync_info = mybir.SyncInfo(on_wait=[w], on_update=[])
                        insts.insert(i, nop)
                        i += 1
                i += 1


def _build_program(s_len, tmin):
    """Build the per-core Bass program. Identical on all cores (SPMD)."""
    nc = bass.Bass('TRN2', target_bir_lowering=False, debug=False)

    ftime_d = nc.dram_tensor('ftime', [BL, s_len * NT], F16, kind='ExternalInput').ap()
    rel0_d = nc.dram_tensor('rel0', [BL, NT], F16, kind='ExternalInput').ap()
    eqt8_d = nc.dram_tensor('eqt8', [BL, s_len], I8, kind='ExternalInput').ap()
    actf_d = nc.dram_tensor('actf', [BL, s_len], F32, kind='ExternalInput').ap()
    trep_d = nc.dram_tensor('trep', [BL, NT * NT], F16, kind='ExternalInput').ap()
    tstop_d = nc.dram_tensor('tstop', [BL, NT], F16, kind='ExternalInput').ap()
    iota_d = nc.dram_tensor('iotau', [BL, NT], mybir.dt.uint32,
                            kind='ExternalInput').ap()
    iota16_d = nc.dram_tensor('iotamb16', [BL, NT], F16, kind='ExternalInput').ap()
    ident_d = nc.dram_tensor('ident', [BL, BL], BF16, kind='ExternalInput').ap()
    tbf_d = nc.dram_tensor('tbf', [NT, NT], BF16, kind='ExternalInput').ap()
    dec_d = nc.dram_tensor('dec', [BL, s_len], I32, kind='ExternalOutput').ap()
    dbga_d = nc.dram_tensor('dbga', [BL, s_len * NT], F16,
                            kind='ExternalOutput').ap() if _DEBUG_DUMP else None
    dbgc_d = nc.dram_tensor('dbgc', [BL, s_len], F32,
                            kind='ExternalOutput').ap() if _DEBUG_DUMP else None

    with tile.TileContext(nc) as tc:
        with tc.tile_pool(name='res', bufs=1) as res, \
             tc.tile_pool(name='fch', bufs=4) as fpool, \
             tc.tile_pool(name='cbtmp', bufs=2) as cbpool, \
             tc.tile_pool(name='tmp', bufs=3) as tmp, \
             tc.tile_pool(name='ps', bufs=4, space='PSUM') as psum:

            # ---- resident constants & state ----
            trep = res.tile([BL, NT * NT], F16, tag='trep')
            nc.gpsimd.dma_start(trep[:], trep_d[:])
            tstop = res.tile([BL, NT], F16, tag='tstop')
            nc.gpsimd.dma_start(tstop[:], tstop_d[:])
            iota = res.tile([BL, NT], mybir.dt.uint32, tag='iota')
            iota16 = res.tile([BL, NT], F16, tag='iota16')
            ident = res.tile([BL, BL], BF16, tag='ident')
            tbf = res.tile([NT, NT], BF16, tag='tbf')
            eqt8 = res.tile([BL, s_len], I8, tag='eqt8')
            actf = res.tile([BL, s_len], F32, tag='actf')

            def _late_dmas():
                # only needed by cbpre/traceback; issued behind the forward
                # feats chunks so the first add isn't serialized behind them
                nc.gpsimd.dma_start(iota[:], iota_d[:])
                nc.gpsimd.dma_start(iota16[:], iota16_d[:])
                nc.gpsimd.dma_start(ident[:], ident_d[:])
                nc.gpsimd.dma_start(tbf[:], tbf_d[:])
                nc.gpsimd.dma_start(eqt8[:], eqt8_d[:])
                nc.gpsimd.dma_start(actf[:], actf_d[:])

            AST = 52                       # padded step stride (4B-aligned)
            ahist = res.tile([BL, s_len * AST], F16, tag='ahist')
            nc.gpsimd.dma_start(ahist[:, 0:NT], rel0_d[:])

            s16 = res.tile([BL, NT * NT], F16, tag='s16')
            h16 = res.tile([BL, NT * 25], F16, tag='h16')
            rel16 = res.tile([BL, NT], F16, tag='rel16')
            nc.vector.tensor_copy(rel16[:], ahist[:, 0:NT])
            dm = res.tile([BL, 1], F32, tag='dm')
            decall8 = res.tile([BL, s_len * 8], mybir.dt.uint32,
                               tag='decall8')
            nc.vector.memset(decall8[:], 0.0)
            cballu = res.tile([BL, s_len], mybir.dt.uint32, tag='cballu')
            cball16 = res.tile([BL, s_len], F16, tag='cball16')
            mall16 = res.tile([BL, s_len], F16, tag='mall16')

            # ---- forward (all fp16 on DVE; re-center every RC steps) ----
            RC = 8
            fwd_scope = nc.named_scope('fwd')
            fwd_scope.__enter__()
            n_ch = (s_len + FCH - 1) // FCH
            for c in range(n_ch):
                t0 = c * FCH
                t1 = min(t0 + FCH, s_len)
                ft = fpool.tile([BL, (t1 - t0) * NT], F16, tag='fch')
                nc.gpsimd.dma_start(ft[:], ftime_d[:, t0 * NT:t1 * NT])
                if c == 0:
                    _late_dmas()
                for t in range(max(t0, 1), t1):
                    # previous alphas: re-centered copy on RC boundaries,
                    # else the raw ahist slice (argmax is shift-invariant)
                    prev = rel16[:] if (t - 1) % RC == 0 \
                        else ahist[:, (t - 1) * AST:(t - 1) * AST + NT]
                    nc.vector.tensor_tensor(
                        s16[:].rearrange('p (j i) -> p j i', j=NT),
                        trep[:].rearrange('p (j i) -> p j i', j=NT),
                        prev.unsqueeze(1).broadcast_to([BL, NT, NT]),
                        op=_AluOp.add)
                    # split-combine: tt-max runs at 2 elem/cycle (fp16 dual
                    # pump) while reduce is 1/cycle; halve the reduce's input
                    s3 = s16[:].rearrange('p (j i) -> p j i', j=NT)
                    nc.vector.tensor_tensor(
                        h16[:].rearrange('p (j i) -> p j i', j=NT),
                        s3[:, :, 0:25], s3[:, :, 25:50], op=_AluOp.max)
                    red = tmp.tile([BL, NT], F16, tag='red')
                    nc.vector.reduce_max(
                        red[:], h16[:].rearrange('p (j i) -> p j i', j=NT),
                        axis=_Axis.X)
                    # ahist_t = red + f_t (fp16)
                    nc.vector.tensor_tensor(
                        ahist[:, t * AST:t * AST + NT], red[:],
                        ft[:, (t - t0) * NT:(t - t0 + 1) * NT], op=_AluOp.add)
                    if t % RC == 0:
                        nc.vector.reduce_max(
                            dm[:], ahist[:, t * AST:t * AST + NT], axis=_Axis.X)
                        nc.vector.tensor_scalar(
                            rel16[:], in0=ahist[:, t * AST:t * AST + NT],
                            scalar1=dm[:], scalar2=None, op0=_AluOp.subtract)

            fwd_scope.__exit__(None, None, None)
            cb_scope = nc.named_scope('cbpre')
            cb_scope.__enter__()
            # ---- best-last candidates (fp16), in 8-step micro-chunks ----
            # Emitted lazily: each chunk becomes 6 small DVE ops that slot
            # into the traceback's PE-leg idle windows (one op per tb step)
            # instead of serializing with the DVE-saturated forward phase.
            CBC = 8

            def _cb_chunk_ops(t0):
                tc_n = min(CBC, s_len - t0)
                av = ahist[:, t0 * AST:(t0 + tc_n) * AST].rearrange(
                    'p (t i) -> p t i', t=tc_n)[:, :, 0:NT]
                cs = cbpool.tile([BL, CBC * NT], F16, tag='cs')
                csv = cs[:, 0:tc_n * NT].rearrange('p (t i) -> p t i', t=tc_n)
                q = cbpool.tile([BL, CBC * NT], F16, tag='q')
                qv = q[:, 0:tc_n * NT].rearrange('p (t i) -> p t i', t=tc_n)
                yield lambda: nc.vector.tensor_tensor(
                    csv, av, tstop[:].unsqueeze(1).broadcast_to([BL, tc_n, NT]),
                    op=_AluOp.add)
                yield lambda: nc.vector.reduce_max(
                    mall16[:, t0:t0 + tc_n], csv, axis=_Axis.X)
                yield lambda: nc.vector.tensor_tensor(
                    qv, csv,
                    mall16[:, t0:t0 + tc_n].unsqueeze(2).broadcast_to(
                        [BL, tc_n, NT]),
                    op=_AluOp.is_equal)
                yield lambda: nc.vector.tensor_tensor(
                    csv, qv, iota16[:].unsqueeze(1).broadcast_to([BL, tc_n, NT]),
                    op=_AluOp.mult)
                yield lambda: nc.vector.tensor_reduce(
                    cball16[:, t0:t0 + tc_n], csv, axis=_Axis.X, op=_AluOp.min)
                yield lambda: nc.vector.tensor_scalar(
                    cballu[:, t0:t0 + tc_n], in0=cball16[:, t0:t0 + tc_n],
                    scalar1=BIGF, scalar2=None, op0=_AluOp.add)

            chunk_starts = [t0 for t0 in range(0, s_len, CBC)
                            if t0 + min(CBC, s_len - t0) > tmin]
            chunk_starts.sort(reverse=True)        # tb consumes high t first
            # chunks for the top 16 timesteps must complete before tb starts
            n_upfront = 0
            while n_upfront < len(chunk_starts) and \
                    chunk_starts[n_upfront] + CBC > s_len - 16:
                n_upfront += 1
            for t0 in chunk_starts[:n_upfront]:
                for op in _cb_chunk_ops(t0):
                    op()
            cb_stream = []
            for t0 in chunk_starts[n_upfront:]:
                cb_stream.extend(_cb_chunk_ops(t0))
            cb_stream.reverse()                    # pop() yields in order

            decf = res.tile([BL, s_len], F32, tag='decf')
            deci = res.tile([BL, s_len], I32, tag='deci')

            def _fin_chunk_ops(t0n):
                dv = decall8[:, 8 * t0n:8 * (t0n + 64)].rearrange(
                    'p (t e) -> p t e', e=8)[:, :, 0:1].rearrange(
                    'p t e -> p (t e)')
                yield lambda: nc.vector.tensor_tensor(
                    decf[:, t0n:t0n + 64], dv, actf[:, t0n:t0n + 64],
                    op=_AluOp.mult)

                def _cvt_dma():
                    nc.vector.tensor_copy(deci[:, t0n:t0n + 64],
                                          decf[:, t0n:t0n + 64])
                    nc.gpsimd.dma_start(dec_d[:, t0n:t0n + 64],
                                        deci[:, t0n:t0n + 64])
                yield _cvt_dma

            fin_stream = []
            for cc in range(s_len // 64 - 1, 0, -1):
                for op in _fin_chunk_ops(64 * cc):
                    fin_stream.append((64 * cc - 1, op))
            fin_stream.reverse()              # pop() yields descending cc

            cb_scope.__exit__(None, None, None)
            tb_scope = nc.named_scope('tb')
            tb_scope.__enter__()
            # ---- traceback: decall8[:, 8t] holds the u32 ptr/tag ----
            # min sequence length is S//4, so no reset fires below tmin
            for t in range(s_len - 1, -1, -1):
                if t >= tmin:
                    nc.vector.copy_predicated(decall8[:, 8 * t:8 * t + 1],
                                              eqt8[:, t:t + 1],
                                              cballu[:, t:t + 1])
                if t == 0:
                    break
                # one-hot of current pointer -> PE transpose -> one bf16
                # matmul gathers tcol = T_bf16[:, ptr]
                oh = tmp.tile([BL, NT], BF16, tag='oh')
                nc.vector.tensor_tensor(
                    oh[:], iota[:],
                    decall8[:, 8 * t:8 * t + 1].broadcast_to([BL, NT]),
                    op=_AluOp.is_equal)
                ohT_ps = psum.tile([NT, BL], BF16, tag='ohT')
                nc.tensor.transpose(ohT_ps[:], oh[:], ident[:])
                ohT = tmp.tile([NT, BL], BF16, tag='ohTs')
                nc.vector.tensor_copy(ohT[:], ohT_ps[:])
                tcol_ps = psum.tile([BL, NT], F32, tag='tcol')
                nc.tensor.matmul(tcol_ps[:], lhsT=ohT[:], rhs=tbf[:],
                                 start=True, stop=True)
                # s = ahist_{t-1} + tcol; argmax via max8 + max_index
                s = tmp.tile([BL, NT], F32, tag='s')
                nc.vector.tensor_tensor(
                    s[:], ahist[:, (t - 1) * AST:(t - 1) * AST + NT], tcol_ps[:],
                    op=_AluOp.add)
                m8 = tmp.tile([BL, 8], F32, tag='m8')
                nc.vector.max(m8[:], s[:])
                nc.vector.max_index(decall8[:, 8 * (t - 1):8 * t], m8[:], s[:])
                if cb_stream and t <= s_len - 2:
                    cb_stream.pop()()
                elif fin_stream and t <= fin_stream[-1][0]:
                    fin_stream.pop()[1]()

            tb_scope.__exit__(None, None, None)
            # any undripped chunks plus chunk 0
            for _, op in reversed(fin_stream):
                op()
            for op in _fin_chunk_ops(0):
                op()
            if _DEBUG_DUMP:
                nc.gpsimd.dma_start(dbga_d[:], ahist[:, 0:s_len * NT])
                nc.gpsimd.dma_start(dbgc_d[:], cballu[:])

    _split_waits(nc)
    return nc


_CACHE = {}


def _get_program(s_len, tmin):
    key = (s_len, tmin)
    if key not in _CACHE:
        _CACHE[key] = _build_program(s_len, tmin)
    return _CACHE[key]


def kernel(feats, mask, tags, transitions, _trace=False):
    del tags  # unused by Viterbi decode
    feats = np.asarray(feats, dtype=np.float32)
    mask = np.asarray(mask)
    transitions = np.asarray(transitions, dtype=np.float32)
    b, s, tfull = feats.shape
    assert (b, tfull) == (B, TFULL)

    lengths = np.maximum(mask.astype(bool).sum(axis=1), 1).astype(np.int64)  # [B]
    lenm1 = (lengths - 1)[:, None]                                            # [B,1]
    trange = np.arange(s)[None, :]
    eqt8 = (trange == lenm1).astype(np.int8)
    actf = (trange <= lenm1).astype(np.float32)

    import ml_dtypes
    fr = feats[:, :, :NT]                                    # real-tag emissions
    alpha0 = transitions[START, :NT][None, :] + fr[:, 0, :]  # [B, NT] f32
    rel0 = (alpha0 - alpha0.max(axis=1, keepdims=True)).astype(np.float16)
    ftime = np.ascontiguousarray(fr, dtype=np.float16).reshape(B, s * NT)

    transT16 = np.ascontiguousarray(
        transitions[:NT, :NT].T.astype(np.float16))          # [j,i] fp16
    trep = np.ascontiguousarray(
        np.broadcast_to(transT16.reshape(1, NT * NT), (BL, NT * NT)))
    tstop = np.ascontiguousarray(np.broadcast_to(
        transitions[:NT, STOP].astype(np.float16)[None, :], (BL, NT)))
    iotau = np.ascontiguousarray(np.broadcast_to(
        np.arange(NT, dtype=np.uint32)[None, :], (BL, NT)))
    iotamb16 = np.ascontiguousarray(np.broadcast_to(
        (np.arange(NT, dtype=np.float16) - np.float16(BIGF))[None, :],
        (BL, NT)))
    ident = np.eye(BL, dtype=ml_dtypes.bfloat16)
    tbf = np.ascontiguousarray(
        transitions[:NT, :NT].T.astype(ml_dtypes.bfloat16))  # [j,i]: row c = T[:,c]

    tmin = max(0, int(lengths.min()) - 1)
    nc = _get_program(s, tmin)
    in_maps = []
    for c in range(NCORES):
        sl = slice(c * BL, (c + 1) * BL)
        in_maps.append({
            'ftime': ftime[sl], 'rel0': np.ascontiguousarray(rel0[sl]),
            'eqt8': np.ascontiguousarray(eqt8[sl]),
            'actf': np.ascontiguousarray(actf[sl]),
            'trep': trep, 'tstop': tstop, 'iotau': iotau,
            'iotamb16': iotamb16, 'ident': ident, 'tbf': tbf,
        })
    res = run_bass_kernel_spmd(nc, in_maps, list(range(NCORES)), trace=_trace)
    out = np.concatenate([res.results[c]['dec'] for c in range(NCORES)], axis=0)
    if _trace:
        kernel._last_results = res
    return out.astype(np.int32)
